# revision 20
# baseline (speedup 1.0000x reference)
"""3D bilateral filter (window 3, sigma_d=120, sigma_r=1.2) on 8 TRN2 NeuronCores.

Algorithm: factor the range kernel
    exp(-(n-c)^2/a) = phi(n) * phi(c) * exp(2*n*c/a),   phi(x) = exp(-x^2/a)
and approximate exp(2*t/a) on t in [0,1] by a degree-J polynomial
    exp(2t/a) ~= sum_j p_j t^j.
Then with moment fields  phi_j = phi(v) * v^j  and  G_j = conv3x3x3(s, phi_j)
(s = separable spatial Gaussian [alpha,1,alpha] per axis):
    den = phi(c) * sum_j p_j c^j G_j
    num = phi(c) * sum_j p_j c^j G_{j+1}
    out = num / den            (phi(c) cancels)
The 3D conv runs on the Tensor engine: the D-axis (partition dim) conv is a
banded 128x128 matmul (replicate edges folded into the corner entries), and
the 9 (dh,dw) shifts are free-dim AP offsets accumulated in PSUM.  Moment
fields are fp16 (the PE streams fp16 at full rate); recombination keeps its
accumulators in fp32 but forms the c^j * G_j products in fp16 at the DVE's
2x packed rate.

Sharding: 8 cores split H (192 -> 24 rows each) with 1-row halo overlap,
prepared host-side. No cross-core communication.
"""

import sys

for _p in ("/opt/trn_rl_repo",):
    if _p not in sys.path:
        sys.path.insert(0, _p)

import numpy as np

# ---------------- problem constants (hardcoded per spec) ----------------
B, D, H, W = 2, 128, 192, 192
SIGMA_D = 120.0
SIGMA_R = 1.2
A = 2.0 * SIGMA_R * SIGMA_R                 # 2.88
ALPHA = float(np.exp(-1.0 / (2.0 * SIGMA_D * SIGMA_D)))

N_CORES = 8
HPC = H // N_CORES                          # 24 output rows per core
# W layout: [dead, halo, v0..v191, halo, dead] -> interior starts at col 2
# (4-byte aligned for fp16 packed DVE reads)
WW = W + 4                                  # 196
HH = HPC + 2                                # slab rows incl. halo

# tunables
J = 3                                       # polynomial degree for exp(2t/a)
NMOM = J + 2                                # moments G_0..G_{J+1}
CHUNKS = [4, 6, 6, 6, 2]                    # output rows per chunk (sum HPC)
CHMAX = max(CHUNKS)
SUBROWS = 2                                 # rows per PSUM sub-chunk (<=512 fp32 bank)
PRESUM = (3, 4)                             # moments whose W-box-sum runs on DMA


def _fit_poly(deg):
    # least-squares fit of exp(2t/A) at Chebyshev nodes on [0,1]
    t = (np.cos(np.pi * (np.arange(4000) + 0.5) / 4000) + 1.0) / 2.0
    y = np.exp(2.0 * t / A)
    V = np.vander(t, deg + 1, increasing=True)
    p, *_ = np.linalg.lstsq(V, y, rcond=None)
    return [float(c) for c in p]


PCOEF = _fit_poly(J)


def _band_matrices():
    """D-axis conv band matrix (replicate-edge corners) x 3 spatial scales."""
    b0 = np.zeros((128, 128), np.float64)
    for i in range(128):
        b0[i, i] = 1.0
        if i > 0:
            b0[i - 1, i] = ALPHA
        if i < 127:
            b0[i + 1, i] = ALPHA
    b0[0, 0] += ALPHA
    b0[127, 127] += ALPHA
    bands = np.concatenate(
        [b0, ALPHA * b0, (ALPHA * ALPHA) * b0], axis=1
    )  # [128, 384]
    return bands.astype(np.float32)


_COMPILED = None


def _build():
    import concourse.bacc as bacc
    import concourse.mybir as mybir
    import concourse.tile as tile

    f32 = mybir.dt.float32
    f16 = mybir.dt.float16
    AF = mybir.ActivationFunctionType
    OP = mybir.AluOpType

    nc = bacc.Bacc("TRN2", target_bir_lowering=False, debug=False)
    vol = nc.dram_tensor("vol", [B, D, HH, WW], f32, kind="ExternalInput")
    bands = nc.dram_tensor("bands", [128, 3 * 128], f32, kind="ExternalInput")
    out = nc.dram_tensor("out", [B, D, HPC, W], f32, kind="ExternalOutput")

    FSLAB = HH * WW
    HRMAX = CHMAX + 2
    FHALO = HRMAX * WW              # free size of halo-extent (phi) tiles
    FOUT = CHMAX * W                # free size of output-extent tiles
    FSUB = SUBROWS * W              # free size of one PSUM sub-chunk

    with tile.TileContext(nc) as tc:
        with tc.tile_pool(name="const", bufs=1) as cpool, \
             tc.tile_pool(name="slab", bufs=2) as spool, \
             tc.tile_pool(name="sbuf", bufs=2) as pool, \
             tc.tile_pool(name="gpool", bufs=2) as gpool, \
             tc.tile_pool(name="hpool", bufs=1) as hpool, \
             tc.tile_pool(name="psum", bufs=8, space="PSUM") as psum:

            bf = cpool.tile([128, 3 * 128], f32, tag="bands_f32")
            nc.sync.dma_start(bf[:, :], bands.ap())
            bmm = cpool.tile([128, 3 * 128], f16, tag="bands_mm")
            nc.vector.tensor_copy(bmm[:, :], bf[:, :])
            bmats = [bmm[:, 128 * m:128 * (m + 1)] for m in range(3)]

            # (dh, dw) -> band matrix index by dh^2+dw^2
            offsets = [(dh, dw) for dh in (-1, 0, 1) for dw in (-1, 0, 1)]

            def emit_recombine(gt, v16v, b, r0, ch):
                """num/den polynomial combine for one finished chunk."""
                fo = ch * W
                cap16 = v16v[:, 1:1 + ch, 2:2 + W]     # fp16 center values
                c2 = hpool.tile([128, FOUT], f16, tag="c2")
                c3 = hpool.tile([128, FOUT], f16, tag="c3")
                nc.vector.tensor_tensor(c2[:, :fo], cap16, cap16, op=OP.mult)
                nc.vector.tensor_tensor(c3[:, :fo], c2[:, :fo], cap16, op=OP.mult)
                cpow = [None, cap16, c2, c3]

                xd = hpool.tile([128, FOUT], f32, tag="xd")
                xn = hpool.tile([128, FOUT], f32, tag="xn")
                nc.scalar.mul(xd[:, :fo], gt[0][:, :fo], PCOEF[0])
                nc.scalar.mul(xn[:, :fo], gt[1][:, :fo], PCOEF[0])
                tprod = hpool.tile([128, FOUT], f16, tag="tprod")
                for j in range(1, J + 1):
                    cj = cpow[j] if j == 1 else cpow[j][:, :fo]
                    nc.vector.tensor_tensor(
                        tprod[:, :fo], cj, gt[j][:, :fo], op=OP.mult)
                    nc.vector.scalar_tensor_tensor(
                        xd[:, :fo], tprod[:, :fo], PCOEF[j], xd[:, :fo],
                        op0=OP.mult, op1=OP.add)
                    nc.vector.tensor_tensor(
                        tprod[:, :fo], cj, gt[j + 1][:, :fo], op=OP.mult)
                    nc.vector.scalar_tensor_tensor(
                        xn[:, :fo], tprod[:, :fo], PCOEF[j], xn[:, :fo],
                        op0=OP.mult, op1=OP.add)

                # out = xn / xd  (xd in [14, 28] — approx recip is safe)
                rc = hpool.tile([128, FOUT], f32, tag="rc")
                nc.vector.reciprocal_approx_fast(out=rc[:, :fo], in_=xd[:, :fo])
                ot = pool.tile([128, FOUT], f32, tag="ot")
                nc.vector.tensor_tensor(ot[:, :fo], xn[:, :fo], rc[:, :fo],
                                        op=OP.mult)
                nc.sync.dma_start(out.ap()[b, :, r0:r0 + ch, :], ot[:, :fo])

            pending = None
            for b in range(B):
                bsl = spool.tile([128, FSLAB], f32, tag="bslab")
                bounds = [0, CHUNKS[0] + 2, 14, 20, HH]
                for ra, rb in zip(bounds, bounds[1:]):
                    nc.sync.dma_start(bsl[:, ra * WW:rb * WW],
                                      vol.ap()[b, :, ra:rb, :])
                bslv = bsl[:, :].rearrange("p (r w) -> p r w", r=HH)

                r0 = 0
                for ich, ch in enumerate(CHUNKS):
                    hr = ch + 2
                    vch = bslv[:, r0:r0 + hr, :]

                    # chunk-extent moment fields phi_j = exp(-v^2/A)*v^j (fp16)
                    v16 = pool.tile([128, FHALO], f16, tag="v16")
                    nc.scalar.copy(v16[:, :hr * WW], vch)
                    v16v = v16[:, :hr * WW].rearrange("p (r w) -> p r w", r=hr)
                    phis = []
                    ph0 = pool.tile([128, FHALO], f16, tag="phi0")
                    nc.scalar.activation(ph0[:, :hr * WW], vch, AF.Square)
                    nc.scalar.activation(ph0[:, :hr * WW], ph0[:, :hr * WW],
                                         AF.Exp, scale=-1.0 / A)
                    phis.append(ph0)
                    for j in range(1, NMOM):
                        pj = pool.tile([128, FHALO], f16, tag=f"phi{j}",
                                       name=f"phi{j}_{b}_{ich}")
                        nc.vector.tensor_tensor(
                            pj[:, :hr * WW], phis[-1][:, :hr * WW],
                            v16[:, :hr * WW], op=OP.mult)
                        phis.append(pj)
                    phivs = [p[:, :hr * WW].rearrange("p (r w) -> p r w", r=hr)
                             for p in phis]

                    # W-axis box pre-sum on the DMA engines (alpha==1 in fp16)
                    # for PRESUM moments: their conv then needs only the 3
                    # dh-offset matmuls instead of 9.
                    psiv = {}
                    for j in PRESUM:
                        psi = pool.tile([128, FHALO], f16, tag=f"psi{j}",
                                        name=f"psi{j}_{b}_{ich}")
                        pv = psi[:, :hr * WW].rearrange(
                            "p (r w) -> p r w", r=hr)
                        dst = pv[:, 0:hr, 2:2 + W]
                        nc.gpsimd.dma_start(dst, phivs[j][:, 0:hr, 1:1 + W])
                        nc.gpsimd.dma_start(dst, phivs[j][:, 0:hr, 2:2 + W],
                                            accum_op=OP.add)
                        nc.gpsimd.dma_start(dst, phivs[j][:, 0:hr, 3:3 + W],
                                            accum_op=OP.add)
                        psiv[j] = pv

                    # G_0, G_1 carry the dominant polynomial terms — keep them
                    # fp32; higher moments tolerate fp16.
                    gt = [gpool.tile([128, FOUT], f32 if j <= 1 else f16,
                                     tag=f"G{j}", name=f"G{j}_{b}_{ich}")
                          for j in range(NMOM)]
                    for j in range(NMOM):
                        if j in PRESUM:
                            offs = [(dh, 0) for dh in (-1, 0, 1)]
                            src = psiv[j]
                        else:
                            offs = offsets
                            src = phivs[j]
                        for isub in range(ch // SUBROWS):
                            rr = isub * SUBROWS    # output row within chunk
                            ps = psum.tile([128, FSUB], f32, tag="ps")
                            for k, (dh, dw) in enumerate(offs):
                                m = dh * dh + dw * dw
                                rhs = src[:, rr + 1 + dh: rr + 1 + dh + SUBROWS,
                                          dw + 2: dw + 2 + W]
                                nc.tensor.matmul(
                                    ps[:, :], bmats[m], rhs,
                                    start=(k == 0), stop=(k == len(offs) - 1))
                            nc.scalar.copy(
                                gt[j][:, rr * W:(rr + SUBROWS) * W], ps[:, :])

                    # software pipeline: emit previous chunk's recombination
                    # AFTER this chunk's convs so the PE never waits on the
                    # Vector engine.
                    if pending is not None:
                        emit_recombine(*pending)
                    pending = (gt, v16v, b, r0, ch)
                    r0 += ch

            emit_recombine(*pending)

    nc.compile()
    return nc


def _get_compiled():
    global _COMPILED
    if _COMPILED is None:
        _COMPILED = _build()
    return _COMPILED


def _shard_inputs(volume):
    v = np.asarray(volume)[:, 0]                          # (B, D, H, W)
    vp = np.pad(v, ((0, 0), (0, 0), (1, 1), (2, 2)), mode="edge")
    bands = _band_matrices()
    in_maps = []
    for c in range(N_CORES):
        slab = np.ascontiguousarray(vp[:, :, c * HPC:c * HPC + HH, :])
        in_maps.append({"vol": slab, "bands": bands})
    return in_maps


def _run(volume, trace=False):
    from concourse import bass_utils
    nc = _get_compiled()
    in_maps = _shard_inputs(volume)
    res = bass_utils.run_bass_kernel_spmd(
        nc, in_maps, core_ids=list(range(N_CORES)), trace=trace)
    shards = [res.results[c]["out"] for c in range(N_CORES)]
    full = np.concatenate(shards, axis=2)                 # (B, D, H, W)
    return full[:, None].astype(np.float32), res


def kernel(volume):
    out, _ = _run(volume, trace=False)
    return out


# revision 22
# speedup vs baseline: 1.1592x; 1.1592x over previous
"""3D bilateral filter (window 3, sigma_d=120, sigma_r=1.2) on 8 TRN2 NeuronCores.

Algorithm: factor the range kernel
    exp(-(n-c)^2/a) = phi(n) * phi(c) * exp(2*n*c/a),   phi(x) = exp(-x^2/a)
and approximate exp(2*t/a) on t in [0,1] by a degree-J polynomial
    exp(2t/a) ~= sum_j p_j t^j.
Then with moment fields  phi_j = phi(v) * v^j  and  G_j = conv3x3x3(s, phi_j)
(s = separable spatial Gaussian [alpha,1,alpha] per axis):
    den = phi(c) * sum_j p_j c^j G_j
    num = phi(c) * sum_j p_j c^j G_{j+1}
    out = num / den            (phi(c) cancels)
The 3D conv runs on the Tensor engine: the D-axis (partition dim) conv is a
banded 128x128 matmul (replicate edges folded into the corner entries), and
the 9 (dh,dw) shifts are free-dim AP offsets accumulated in PSUM.  Moment
fields are fp16 (the PE streams fp16 at full rate); recombination keeps its
accumulators in fp32 but forms the c^j * G_j products in fp16 at the DVE's
2x packed rate.

Sharding: 8 cores split H (192 -> 24 rows each) with 1-row halo overlap,
prepared host-side. No cross-core communication.
"""

import sys

for _p in ("/opt/trn_rl_repo",):
    if _p not in sys.path:
        sys.path.insert(0, _p)

import numpy as np

# ---------------- problem constants (hardcoded per spec) ----------------
B, D, H, W = 2, 128, 192, 192
SIGMA_D = 120.0
SIGMA_R = 1.2
A = 2.0 * SIGMA_R * SIGMA_R                 # 2.88
ALPHA = float(np.exp(-1.0 / (2.0 * SIGMA_D * SIGMA_D)))

N_CORES = 8
HPC = H // N_CORES                          # 24 output rows per core
# W layout: [dead, halo, v0..v191, halo, dead] -> interior starts at col 2
# (4-byte aligned for fp16 packed DVE reads)
WW = W + 4                                  # 196
HH = HPC + 2                                # slab rows incl. halo

# tunables
J = 3                                       # polynomial degree for exp(2t/a)
NMOM = J + 2                                # moments G_0..G_{J+1}
CHUNKS = [6, 8, 8, 2]                       # output rows per chunk (sum HPC)
CHMAX = max(CHUNKS)
SUBROWS = 2                                 # rows per PSUM sub-chunk (<=512 fp32 bank)
PRESUM = ()                                 # moments whose W-box-sum runs on DMA


def _fit_poly(deg):
    # least-squares fit of exp(2t/A) at Chebyshev nodes on [0,1]
    t = (np.cos(np.pi * (np.arange(4000) + 0.5) / 4000) + 1.0) / 2.0
    y = np.exp(2.0 * t / A)
    V = np.vander(t, deg + 1, increasing=True)
    p, *_ = np.linalg.lstsq(V, y, rcond=None)
    return [float(c) for c in p]


PCOEF = _fit_poly(J)


def _band_matrices():
    """D-axis conv band matrix (replicate-edge corners) x 3 spatial scales."""
    b0 = np.zeros((128, 128), np.float64)
    for i in range(128):
        b0[i, i] = 1.0
        if i > 0:
            b0[i - 1, i] = ALPHA
        if i < 127:
            b0[i + 1, i] = ALPHA
    b0[0, 0] += ALPHA
    b0[127, 127] += ALPHA
    bands = np.concatenate(
        [b0, ALPHA * b0, (ALPHA * ALPHA) * b0], axis=1
    )  # [128, 384]
    return bands.astype(np.float32)


_COMPILED = None


def _build():
    import concourse.bacc as bacc
    import concourse.mybir as mybir
    import concourse.tile as tile

    f32 = mybir.dt.float32
    f16 = mybir.dt.float16
    AF = mybir.ActivationFunctionType
    OP = mybir.AluOpType

    nc = bacc.Bacc("TRN2", target_bir_lowering=False, debug=False)
    vol = nc.dram_tensor("vol", [B, D, HH, WW], f32, kind="ExternalInput")
    bands = nc.dram_tensor("bands", [128, 3 * 128], f32, kind="ExternalInput")
    out = nc.dram_tensor("out", [B, D, HPC, W], f32, kind="ExternalOutput")

    FSLAB = HH * WW
    HRMAX = CHMAX + 2
    FHALO = HRMAX * WW              # free size of halo-extent (phi) tiles
    FOUT = CHMAX * W                # free size of output-extent tiles
    FSUB = SUBROWS * W              # free size of one PSUM sub-chunk

    with tile.TileContext(nc) as tc:
        with tc.tile_pool(name="const", bufs=1) as cpool, \
             tc.tile_pool(name="slab", bufs=2) as spool, \
             tc.tile_pool(name="sbuf", bufs=2) as pool, \
             tc.tile_pool(name="gpool", bufs=2) as gpool, \
             tc.tile_pool(name="hpool", bufs=1) as hpool, \
             tc.tile_pool(name="psum", bufs=8, space="PSUM") as psum:

            bf = cpool.tile([128, 3 * 128], f32, tag="bands_f32")
            nc.sync.dma_start(bf[:, :], bands.ap())
            bmm = cpool.tile([128, 3 * 128], f16, tag="bands_mm")
            nc.vector.tensor_copy(bmm[:, :], bf[:, :])
            bmats = [bmm[:, 128 * m:128 * (m + 1)] for m in range(3)]

            # (dh, dw) -> band matrix index by dh^2+dw^2
            offsets = [(dh, dw) for dh in (-1, 0, 1) for dw in (-1, 0, 1)]

            def emit_recombine(gt, v16v, b, r0, ch):
                """num/den polynomial combine for one finished chunk."""
                fo = ch * W
                cap16 = v16v[:, 1:1 + ch, 2:2 + W]     # fp16 center values
                c2 = hpool.tile([128, FOUT], f16, tag="c2")
                c3 = hpool.tile([128, FOUT], f16, tag="c3")
                nc.vector.tensor_tensor(c2[:, :fo], cap16, cap16, op=OP.mult)
                nc.vector.tensor_tensor(c3[:, :fo], c2[:, :fo], cap16, op=OP.mult)
                cpow = [None, cap16, c2, c3]

                xd = hpool.tile([128, FOUT], f32, tag="xd")
                xn = hpool.tile([128, FOUT], f32, tag="xn")
                nc.scalar.mul(xd[:, :fo], gt[0][:, :fo], PCOEF[0])
                nc.scalar.mul(xn[:, :fo], gt[1][:, :fo], PCOEF[0])
                # products c^j * G in fp16 (2x packed rate); the two small
                # high-order terms pair up in fp16 first (their sum is ~10%
                # of the total, so the fp16 rounding there is harmless).
                t1 = hpool.tile([128, FOUT], f16, tag="t1")
                t2 = hpool.tile([128, FOUT], f16, tag="t2")
                t3 = hpool.tile([128, FOUT], f16, tag="t3")
                for xacc, goff in ((xd, 0), (xn, 1)):
                    nc.vector.tensor_tensor(
                        t1[:, :fo], cpow[1], gt[1 + goff][:, :fo], op=OP.mult)
                    nc.vector.tensor_tensor(
                        t2[:, :fo], cpow[2][:, :fo], gt[2 + goff][:, :fo],
                        op=OP.mult)
                    nc.vector.tensor_tensor(
                        t3[:, :fo], cpow[3][:, :fo], gt[3 + goff][:, :fo],
                        op=OP.mult)
                    # s23 = t2 + (p3/p2) t3   (fp16, 2x)
                    nc.vector.scalar_tensor_tensor(
                        t3[:, :fo], t3[:, :fo], PCOEF[3] / PCOEF[2],
                        t2[:, :fo], op0=OP.mult, op1=OP.add)
                    nc.vector.scalar_tensor_tensor(
                        xacc[:, :fo], t1[:, :fo], PCOEF[1], xacc[:, :fo],
                        op0=OP.mult, op1=OP.add)
                    nc.vector.scalar_tensor_tensor(
                        xacc[:, :fo], t3[:, :fo], PCOEF[2], xacc[:, :fo],
                        op0=OP.mult, op1=OP.add)

                # out = xn / xd  (xd in [14, 28] — approx recip is safe)
                rc = hpool.tile([128, FOUT], f32, tag="rc")
                nc.vector.reciprocal_approx_fast(out=rc[:, :fo], in_=xd[:, :fo])
                ot = pool.tile([128, FOUT], f32, tag="ot")
                nc.vector.tensor_tensor(ot[:, :fo], xn[:, :fo], rc[:, :fo],
                                        op=OP.mult)
                nc.sync.dma_start(out.ap()[b, :, r0:r0 + ch, :], ot[:, :fo])

            pending = None
            for b in range(B):
                bsl = spool.tile([128, FSLAB], f32, tag="bslab")
                bounds = [0, CHUNKS[0] + 2, 14, 20, HH]
                for ra, rb in zip(bounds, bounds[1:]):
                    nc.sync.dma_start(bsl[:, ra * WW:rb * WW],
                                      vol.ap()[b, :, ra:rb, :])
                bslv = bsl[:, :].rearrange("p (r w) -> p r w", r=HH)

                r0 = 0
                for ich, ch in enumerate(CHUNKS):
                    hr = ch + 2
                    vch = bslv[:, r0:r0 + hr, :]

                    # chunk-extent moment fields phi_j = exp(-v^2/A)*v^j (fp16)
                    v16 = pool.tile([128, FHALO], f16, tag="v16")
                    nc.scalar.copy(v16[:, :hr * WW], vch)
                    v16v = v16[:, :hr * WW].rearrange("p (r w) -> p r w", r=hr)
                    phis = []
                    ph0 = pool.tile([128, FHALO], f16, tag="phi0")
                    nc.scalar.activation(ph0[:, :hr * WW], vch, AF.Square)
                    nc.scalar.activation(ph0[:, :hr * WW], ph0[:, :hr * WW],
                                         AF.Exp, scale=-1.0 / A)
                    phis.append(ph0)
                    for j in range(1, NMOM):
                        pj = pool.tile([128, FHALO], f16, tag=f"phi{j}",
                                       name=f"phi{j}_{b}_{ich}")
                        nc.vector.tensor_tensor(
                            pj[:, :hr * WW], phis[-1][:, :hr * WW],
                            v16[:, :hr * WW], op=OP.mult)
                        phis.append(pj)
                    phivs = [p[:, :hr * WW].rearrange("p (r w) -> p r w", r=hr)
                             for p in phis]

                    # W-axis box pre-sum on the DMA engines (alpha==1 in fp16)
                    # for PRESUM moments: their conv then needs only the 3
                    # dh-offset matmuls instead of 9.
                    psiv = {}
                    for j in PRESUM:
                        psi = pool.tile([128, FHALO], f16, tag=f"psi{j}",
                                        name=f"psi{j}_{b}_{ich}")
                        pv = psi[:, :hr * WW].rearrange(
                            "p (r w) -> p r w", r=hr)
                        dst = pv[:, 0:hr, 2:2 + W]
                        nc.gpsimd.dma_start(dst, phivs[j][:, 0:hr, 1:1 + W])
                        nc.gpsimd.dma_start(dst, phivs[j][:, 0:hr, 2:2 + W],
                                            accum_op=OP.add)
                        nc.gpsimd.dma_start(dst, phivs[j][:, 0:hr, 3:3 + W],
                                            accum_op=OP.add)
                        psiv[j] = pv

                    # G_0, G_1 carry the dominant polynomial terms — keep them
                    # fp32; higher moments tolerate fp16.
                    gt = [gpool.tile([128, FOUT], f32 if j <= 1 else f16,
                                     tag=f"G{j}", name=f"G{j}_{b}_{ich}")
                          for j in range(NMOM)]
                    for j in range(NMOM):
                        if j in PRESUM:
                            offs = [(dh, 0) for dh in (-1, 0, 1)]
                            src = psiv[j]
                        else:
                            offs = offsets
                            src = phivs[j]
                        for isub in range(ch // SUBROWS):
                            rr = isub * SUBROWS    # output row within chunk
                            ps = psum.tile([128, FSUB], f32, tag="ps")
                            for k, (dh, dw) in enumerate(offs):
                                m = dh * dh + dw * dw
                                rhs = src[:, rr + 1 + dh: rr + 1 + dh + SUBROWS,
                                          dw + 2: dw + 2 + W]
                                nc.tensor.matmul(
                                    ps[:, :], bmats[m], rhs,
                                    start=(k == 0), stop=(k == len(offs) - 1))
                            nc.scalar.copy(
                                gt[j][:, rr * W:(rr + SUBROWS) * W], ps[:, :])

                    # software pipeline: emit previous chunk's recombination
                    # AFTER this chunk's convs so the PE never waits on the
                    # Vector engine.
                    if pending is not None:
                        emit_recombine(*pending)
                    pending = (gt, v16v, b, r0, ch)
                    r0 += ch

            emit_recombine(*pending)

    nc.compile()
    return nc


def _get_compiled():
    global _COMPILED
    if _COMPILED is None:
        _COMPILED = _build()
    return _COMPILED


def _shard_inputs(volume):
    v = np.asarray(volume)[:, 0]                          # (B, D, H, W)
    vp = np.pad(v, ((0, 0), (0, 0), (1, 1), (2, 2)), mode="edge")
    bands = _band_matrices()
    in_maps = []
    for c in range(N_CORES):
        slab = np.ascontiguousarray(vp[:, :, c * HPC:c * HPC + HH, :])
        in_maps.append({"vol": slab, "bands": bands})
    return in_maps


def _run(volume, trace=False):
    from concourse import bass_utils
    nc = _get_compiled()
    in_maps = _shard_inputs(volume)
    res = bass_utils.run_bass_kernel_spmd(
        nc, in_maps, core_ids=list(range(N_CORES)), trace=trace)
    shards = [res.results[c]["out"] for c in range(N_CORES)]
    full = np.concatenate(shards, axis=2)                 # (B, D, H, W)
    return full[:, None].astype(np.float32), res


def kernel(volume):
    out, _ = _run(volume, trace=False)
    return out
